# revision 3
# baseline (speedup 1.0000x reference)
"""FNO2d U-Net forward for Trainium2 (8 NeuronCores visible, batch=4
data-parallel over 4 cores).

The reference's rfft2/complex einsums do not lower through neuronx-cc, so
every FFT is rewritten as small real DFT matmuls over the 2m x m retained
modes (real/imag parts split and stacked).  The whole network then lowers
as dense real matmuls + elementwise ops, which XLA compiles for the
NeuronCores.  Falls back to the same math on CPU if device compile fails.
"""
import os
import numpy as np

B, H, W = 4, 256, 256
_CACHE = {}


def _dft_consts(np_, Hc, m1, m2):
    h = np_.arange(Hc)
    klow = np_.arange(m1)
    khigh = np_.arange(Hc - m1, Hc)
    kc = np_.arange(m2)
    ang = lambda k, n, N: -2j * np.pi * np_.outer(k, n) / N
    FrLo = np_.exp(ang(klow, h, Hc))            # [m1, H]
    FrHi = np_.exp(ang(khigh, h, Hc))           # [m1, H]
    Fc = np_.exp(ang(kc, h, Hc)).T              # [W, m2]
    GrLo = np_.exp(-ang(klow, h, Hc)).T         # [H, m1]
    GrHi = np_.exp(-ang(khigh, h, Hc)).T        # [H, m1]
    cl = np_.where(kc == 0, 1.0, 2.0) / (Hc * Hc)
    Gc = cl[:, None] * np_.exp(-ang(kc, h, Hc))  # [m2, W]
    f = lambda a: (a.real.astype(np.float32), a.imag.astype(np.float32))
    return tuple(map(f, (FrLo, FrHi, Fc, GrLo, GrHi, Gc)))


def _make_forward(jnp, weights, dtype):
    cs = {256: _dft_consts(np, 256, 12, 12),
          128: _dft_consts(np, 128, 8, 8),
          64: _dft_consts(np, 64, 4, 4)}

    def cast(a):
        return jnp.asarray(a, dtype)

    def spectral(v, w1, w2, Hc):
        # v: [Ci, Hc, Hc] real; w1/w2: [Ci, Co, m, m, 2]
        (FrLo_r, FrLo_i), (FrHi_r, FrHi_i), (Fc_r, Fc_i), \
            (GrLo_r, GrLo_i), (GrHi_r, GrHi_i), (Gc_r, Gc_i) = cs[Hc]
        e = jnp.einsum
        P_r = e('chw,wl->chl', v, cast(Fc_r))
        P_i = e('chw,wl->chl', v, cast(Fc_i))

        def rowdft(Fr_r, Fr_i):
            xr = e('kh,chl->ckl', cast(Fr_r), P_r) - \
                 e('kh,chl->ckl', cast(Fr_i), P_i)
            xi = e('kh,chl->ckl', cast(Fr_r), P_i) + \
                 e('kh,chl->ckl', cast(Fr_i), P_r)
            return xr, xi

        def mix(xr, xi, w):
            wr = cast(w[..., 0])
            wi = cast(w[..., 1])
            o_r = e('ikl,iokl->okl', xr, wr) - e('ikl,iokl->okl', xi, wi)
            o_i = e('ikl,iokl->okl', xr, wi) + e('ikl,iokl->okl', xi, wr)
            return o_r, o_i

        xlo_r, xlo_i = rowdft(FrLo_r, FrLo_i)
        xhi_r, xhi_i = rowdft(FrHi_r, FrHi_i)
        olo_r, olo_i = mix(xlo_r, xlo_i, w1)
        ohi_r, ohi_i = mix(xhi_r, xhi_i, w2)
        z_r = (e('hk,okl->ohl', cast(GrLo_r), olo_r)
               - e('hk,okl->ohl', cast(GrLo_i), olo_i)
               + e('hk,okl->ohl', cast(GrHi_r), ohi_r)
               - e('hk,okl->ohl', cast(GrHi_i), ohi_i))
        z_i = (e('hk,okl->ohl', cast(GrLo_r), olo_i)
               + e('hk,okl->ohl', cast(GrLo_i), olo_r)
               + e('hk,okl->ohl', cast(GrHi_r), ohi_i)
               + e('hk,okl->ohl', cast(GrHi_i), ohi_r))
        y = e('ohl,lx->ohx', z_r, cast(Gc_r)) - \
            e('ohl,lx->ohx', z_i, cast(Gc_i))
        return y

    def gelu(v):
        from jax.scipy.special import erf
        v32 = v.astype(jnp.float32)
        return (0.5 * v32 * (1.0 + erf(v32 / np.float32(np.sqrt(2.0))))
                ).astype(dtype)

    def conv1x1(v, Wm, b):
        return jnp.einsum('ihw,oi->ohw', v, cast(Wm)) + \
            b.astype(jnp.float32)[:, None, None].astype(dtype)

    def pool(v):
        c, h, w = v.shape
        return v.reshape(c, h // 2, 2, w // 2, 2).mean(axis=(2, 4))

    def up_axis(v, ax):
        v = jnp.moveaxis(v, ax, 0)
        prev = jnp.concatenate([v[:1], v[:-1]], axis=0)
        nxt = jnp.concatenate([v[1:], v[-1:]], axis=0)
        even = 0.25 * prev + 0.75 * v
        odd = 0.75 * v + 0.25 * nxt
        out = jnp.stack([even, odd], axis=1).reshape((-1,) + v.shape[1:])
        return jnp.moveaxis(out, 0, ax)

    def up(v):
        return up_axis(up_axis(v, 1), 2)

    wd = weights

    def fwd(x):
        # x: [H, W, 6] one sample
        x = x.astype(dtype)
        v = jnp.einsum('hwi,oi->ohw', x, cast(wd['fcin_w'])) + \
            cast(wd['fcin_b'][:, None, None])
        x1 = gelu(spectral(v, wd['sc1_w1'], wd['sc1_w2'], 256)
                  + conv1x1(v, wd['c1_w'], wd['c1_b']))
        x1d = pool(x1)
        x2 = gelu(spectral(x1d, wd['sc2_w1'], wd['sc2_w2'], 128)
                  + conv1x1(x1d, wd['c2_w'], wd['c2_b']))
        x2d = pool(x2)
        xb = gelu(spectral(x2d, wd['scb_w1'], wd['scb_w2'], 64)
                  + conv1x1(x2d, wd['cb_w'], wd['cb_b']))
        x2c = jnp.concatenate([up(xb), x2], axis=0)
        x2o = gelu(spectral(x2c, wd['su2_w1'], wd['su2_w2'], 128)
                   + conv1x1(x2c, wd['u2_w'], wd['u2_b']))
        x1c = jnp.concatenate([up(x2o), x1], axis=0)
        x1o = gelu(spectral(x1c, wd['su1_w1'], wd['su1_w2'], 256)
                   + conv1x1(x1c, wd['u1_w'], wd['u1_b']))
        h1 = gelu(jnp.einsum('ihw,oi->ohw', x1o, cast(wd['fc1_w']))
                  + cast(wd['fc1_b'][:, None, None]))
        out = jnp.einsum('ihw,oi->ohw', h1, cast(wd['fc2_w'])) + \
            cast(wd['fc2_b'][:, None, None])
        return jnp.transpose(out, (1, 2, 0)).astype(jnp.float32)

    return fwd


def _get_device_fn(weights):
    """Build (once) the pmapped device function over 4 NeuronCores."""
    import jax
    import jax.numpy as jnp
    if "fn" in _CACHE:
        return _CACHE["fn"]
    devs = [d for d in jax.devices() if d.platform != "cpu"][:B]
    if len(devs) < B:
        raise RuntimeError("not enough neuron devices")
    dtype = jnp.bfloat16 if os.environ.get("FNO_BF16", "1") == "1" \
        else jnp.float32
    fwd = _make_forward(jnp, weights, dtype)
    fn = jax.pmap(fwd, devices=devs)
    _CACHE["fn"] = fn
    return fn


def _cpu_fn(weights):
    import jax
    import jax.numpy as jnp
    cpu = jax.devices("cpu")[0]
    fwd = _make_forward(jnp, weights, jnp.float32)
    return jax.jit(jax.vmap(fwd), device=cpu)


def kernel(**inputs):
    x = np.asarray(inputs["x"], np.float32)
    weights = {k: np.asarray(v) for k, v in inputs.items() if k != "x"}
    try:
        fn = _get_device_fn(weights)
        out = np.asarray(fn(x), np.float32)
        if not np.isfinite(out).all():
            raise RuntimeError("non-finite device output")
        return out
    except Exception:
        _CACHE.pop("fn", None)
        f = _cpu_fn(weights)
        return np.asarray(f(x), np.float32)
